# revision 1
# baseline (speedup 1.0000x reference)
"""Trainium2 Bass kernel for nn_Block_22325240004804 (dense_transformer).

Two-stream cross-attention transformer block, B=8 N=1024 C=768 H=12.
Sharding: pure data parallel — batch element b on core b (no collectives).

Per-core pipeline (one Bass/Tile program), v3:
  P1  LayerNorm(x) -> x_n (DRAM fp32 residual) + x_nb (DRAM bf16, for
      XBAR transpose-loads)
  P2  qkv = x_n @ qkv_wT (bf16, fp32 PSUM, weights fully resident 27.6KB);
      per-head LN over d=64 on the full [128, 2304] row in fp32 (one
      approx-reciprocal per chunk); outputs: qkvb_nat (bf16, one DMA per
      chunk) + q staged fp32 to q2d for the faithful q.reshape residual.
      x_n^T matmul operand comes from 6 XBAR DMA-transpose loads.
  P3  cross attention, scores transposed (S^T[m,n], fp32 PSUM), softmax
      without max-subtraction; denominator via ones-column in the P@V
      matmul; per-head normalization: denom rows moved by DMA to
      partition 0, one approx-reciprocal per head, K=1 fp32r broadcast
      matmul, DVE multiply; q^T/k^T arrive via XBAR DMA-transpose loads;
      proj with K=64 bf16 chunks; residual o = x_n + proj + q_res (fp32);
      LN2 -> x2 bf16 natural to DRAM.
  P4  MLP: x2^T via XBAR transpose-loads; h^T = gelu(fc1) bf16
      (weights stationary, streamed); fc2 back to natural fp32; biases
      via ones-row augmentation when nonzero.

Matmul datapath bf16 (FWL, HAM-warm) with fp32 PSUM accumulation; all
statistics, softmax normalization and residuals in fp32.
"""

import sys

if "/opt/trn_rl_repo" not in sys.path:
    sys.path.insert(0, "/opt/trn_rl_repo")

import numpy as np

B, N, C = 8, 1024, 768
H, HD = 12, 64
S3 = 3 * C          # 2304
HID = 4 * C         # 3072
EPS = 1e-5
P = 128
NCH = N // P        # 8 token chunks
KC = C // P         # 6 contraction chunks over C
NG = S3 // HD       # 36 head-groups per token row
HKC = HID // P      # 24 chunks over HID
QKV_SL = [(0, 512), (512, 512), (1024, 512), (1536, 512), (2048, 256)]

_CACHE = {}


def _build(flags):
    import concourse.bass as bass
    import concourse.tile as tile
    from concourse import bacc, mybir

    f32 = mybir.dt.float32
    f32r = mybir.dt.float32r
    bf16 = mybir.dt.bfloat16
    AF = mybir.ActivationFunctionType
    ALU = mybir.AluOpType
    AX = mybir.AxisListType.X

    (n1_aff, hln_aff, n2_aff, has_projb, has_fc1b, has_fc2b) = flags

    nc = bacc.Bacc("TRN2", target_bir_lowering=False)

    # ---------------- I/O ----------------
    x_in = {
        "b": nc.dram_tensor("x_b", [N, C], f32, kind="ExternalInput"),
        "a": nc.dram_tensor("x_a", [N, C], f32, kind="ExternalInput"),
    }
    qkv_wT = nc.dram_tensor("qkv_wT", [C, S3], bf16, kind="ExternalInput")
    proj_wT = nc.dram_tensor("proj_wT", [C, C], bf16, kind="ExternalInput")
    fc1_wT = nc.dram_tensor("fc1_wT", [C, HID], bf16, kind="ExternalInput")
    fc2_wT = nc.dram_tensor("fc2_wT", [HID, C], bf16, kind="ExternalInput")
    projb_d = nc.dram_tensor("proj_b", [1, C], bf16, kind="ExternalInput") if has_projb else None
    fc1b_d = nc.dram_tensor("fc1_b", [HID], f32, kind="ExternalInput") if has_fc1b else None
    fc2b_d = nc.dram_tensor("fc2_b", [1, C], bf16, kind="ExternalInput") if has_fc2b else None
    n1w_d = nc.dram_tensor("norm1_w", [C], f32, kind="ExternalInput") if n1_aff else None
    n1b_d = nc.dram_tensor("norm1_b", [C], f32, kind="ExternalInput") if n1_aff else None
    n2w_d = nc.dram_tensor("norm2_w", [C], f32, kind="ExternalInput") if n2_aff else None
    n2b_d = nc.dram_tensor("norm2_b", [C], f32, kind="ExternalInput") if n2_aff else None
    hlnw_d = nc.dram_tensor("hln_w", [HD], f32, kind="ExternalInput") if hln_aff else None
    hlnb_d = nc.dram_tensor("hln_b", [HD], f32, kind="ExternalInput") if hln_aff else None
    ones_in = nc.dram_tensor("ones_in", [P], f32r, kind="ExternalInput")
    out_d = {
        "b": nc.dram_tensor("out_b", [N, C], f32, kind="ExternalOutput"),
        "a": nc.dram_tensor("out_a", [N, C], f32, kind="ExternalOutput"),
    }

    with tile.TileContext(nc) as tc:
        with (
            tc.tile_pool(name="dram", bufs=1, space="DRAM") as dram,
            tc.tile_pool(name="const", bufs=1) as const,
            tc.tile_pool(name="s1", bufs=1) as s1,
            tc.tile_pool(name="s1b", bufs=2) as s1b,
            tc.tile_pool(name="s2", bufs=2) as s2,
            tc.tile_pool(name="s3", bufs=3) as s3,
            tc.tile_pool(name="psA", bufs=3, space="PSUM") as psA,
            tc.tile_pool(name="psB", bufs=3, space="PSUM") as psB,
            tc.tile_pool(name="psC", bufs=2, space="PSUM") as psC,
        ):
            # -------- DRAM staging --------
            xn_t = {s: dram.tile([N, C], f32, name=f"xn_{s}", tag=f"xn_{s}") for s in "ba"}
            qkT_t = {s: dram.tile([2 * C, N], bf16, name=f"qkT_{s}", tag=f"qkT_{s}") for s in "ba"}
            v_t = {s: dram.tile([N, C], bf16, name=f"v_{s}", tag=f"v_{s}") for s in "ba"}
            q2d_t = {s: dram.tile([H * N, HD], f32, name=f"q2d_{s}", tag=f"q2d_{s}") for s in "ba"}
            o_t = {s: dram.tile([N, C], f32, name=f"o_{s}", tag=f"o_{s}") for s in "ba"}
            x2T_t = {s: dram.tile([C, N], bf16, name=f"x2T_{s}", tag=f"x2T_{s}") for s in "ba"}

            # -------- constants --------
            from concourse.masks import make_identity
            ident = const.tile([P, P], bf16, tag="ident")
            make_identity(nc, ident)
            ones = const.tile([P, P], f32r, tag="ones")
            _ones_src = ones_in[:]
            nc.gpsimd.dma_start(out=ones, in_=bass.AP(
                tensor=_ones_src.tensor, offset=_ones_src.offset,
                ap=[[0, P]] + list(_ones_src.ap)))
            ones_bf = const.tile([1, P], bf16, tag="ones_bf")
            nc.vector.memset(ones_bf, 1.0)
            epsC = const.tile([P, 1], f32, tag="epsC")
            nc.vector.memset(epsC, EPS)

            if has_projb:
                projb_sb = const.tile([1, C], bf16, tag="projb")
                nc.sync.dma_start(projb_sb, projb_d[:])
            if has_fc2b:
                fc2b_sb = const.tile([1, C], bf16, tag="fc2b")
                nc.sync.dma_start(fc2b_sb, fc2b_d[:])
            if has_fc1b:
                fc1b_sb = const.tile([P, HKC], f32, tag="fc1b")
                nc.sync.dma_start(fc1b_sb, fc1b_d[:].rearrange("(k p) -> p k", p=P))

            def bcast_load(src_ap, cols, tag):
                t = const.tile([P, cols], f32, tag=tag)
                bc = bass.AP(tensor=src_ap.tensor, offset=src_ap.offset,
                             ap=[[0, P]] + list(src_ap.ap))
                nc.gpsimd.dma_start(out=t, in_=bc)
                return t

            if n1_aff:
                n1w_sb = bcast_load(n1w_d[:], C, "n1w")
                n1b_sb = bcast_load(n1b_d[:], C, "n1b")
            if n2_aff:
                n2w_sb = bcast_load(n2w_d[:], C, "n2w")
                n2b_sb = bcast_load(n2b_d[:], C, "n2b")
            if hln_aff:
                hlnw_sb = bcast_load(hlnw_d[:], HD, "hlnw")
                hlnb_sb = bcast_load(hlnb_d[:], HD, "hlnb")

            # -------- helpers --------
            def layernorm_chunk(x_tile, out_tile, w_sb, b_sb):
                """LN over free dim 768 of a [128, 768] tile (fp32)."""
                st = s2.tile([P, 3, 6], f32, tag="lnst")
                for g in range(3):
                    nc.vector.bn_stats(st[:, g, :], x_tile[:, g * 256:(g + 1) * 256])
                mv = s2.tile([P, 2], f32, tag="lnmv")
                nc.vector.bn_aggr(mv, st)
                std = s2.tile([P, 1], f32, tag="lnstd")
                nc.scalar.activation(std, mv[:, 1:2], AF.Sqrt, bias=epsC)
                rstd = s2.tile([P, 1], f32, tag="lnrstd")
                nc.vector.reciprocal(rstd, std)
                nc.vector.tensor_scalar(out_tile, x_tile, mv[:, 0:1], rstd,
                                        ALU.subtract, ALU.mult)
                if w_sb is not None:
                    nc.vector.tensor_tensor(out_tile, out_tile, w_sb, ALU.mult)
                    nc.vector.tensor_tensor(out_tile, out_tile, b_sb, ALU.add)

            def transpose128(src_ap, dst_dram_ap=None, dst_sbuf_ap=None):
                tp = psA.tile([P, P], bf16, tag="psA")
                nc.tensor.transpose(tp, src_ap, ident)
                if dst_sbuf_ap is not None:
                    nc.vector.tensor_copy(dst_sbuf_ap, tp)
                if dst_dram_ap is not None:
                    t = s3.tile([P, P], bf16, tag="tsb")
                    nc.vector.tensor_copy(t, tp)
                    nc.sync.dma_start(dst_dram_ap, t)

            # ============ P1 + P2 per stream ============
            xnTd = {}
            for s in "ba":
                with nc.named_scope(f"p1_{s}"):
                    xnTd[s] = s1.tile([P, KC, N], bf16, name=f"xnT_{s}", tag="xnT")
                    for c in range(NCH):
                        xt = s2.tile([P, C], f32, tag="xin")
                        nc.sync.dma_start(xt, x_in[s][c * P:(c + 1) * P, :])
                        xn = s2.tile([P, C], f32, tag="lnout")
                        layernorm_chunk(xt, xn,
                                        n1w_sb if n1_aff else None,
                                        n1b_sb if n1_aff else None)
                        nc.sync.dma_start(xn_t[s][c * P:(c + 1) * P, :], xn)
                        xnb = s2.tile([P, C], bf16, tag="xnb")
                        nc.vector.tensor_copy(xnb, xn)
                        for t in range(KC):
                            transpose128(
                                xnb[:, t * P:(t + 1) * P],
                                dst_sbuf_ap=xnTd[s][:, t, c * P:(c + 1) * P])

                with nc.named_scope(f"qkv_{s}"):
                    q2d_view = q2d_t[s][:].rearrange("(h n) d -> n h d", h=H)
                    wq = s1.tile([P, KC, S3], bf16, tag="wstream")
                    nc.sync.dma_start(
                        wq, qkv_wT[:].rearrange("(k p) f -> p k f", p=P))
                    xnT = xnTd[s]
                    for c in range(NCH):
                        accs = []
                        for i, (f0, fw) in enumerate(QKV_SL):
                            acc = psA.tile([P, 512], f32, tag="psA", name=f"acc{i}") \
                                if i < 3 else \
                                psB.tile([P, 512], f32, tag="psB", name=f"acc{i}")
                            for k in range(KC):
                                nc.tensor.matmul(
                                    acc[:, :fw],
                                    xnT[:, k, c * P:(c + 1) * P],
                                    wq[:, k, f0:f0 + fw],
                                    start=(k == 0), stop=(k == KC - 1))
                            accs.append(acc)
                        qsb = s2.tile([P, S3], f32, tag="qsb")
                        sumsq = s2.tile([P, NG], f32, tag="hsumsq")
                        for i, (f0, fw) in enumerate(QKV_SL):
                            nc.scalar.copy(qsb[:, f0:f0 + fw], accs[i][:, :fw])
                            sqp = psC.tile([P, 512], f32, tag="psC")
                            nc.scalar.activation(sqp[:, :fw], accs[i][:, :fw],
                                                 AF.Square)
                            nc.vector.reduce_sum(
                                sumsq[:, f0 // HD:(f0 + fw) // HD],
                                sqp[:, :fw].rearrange("p (g d) -> p g d", d=HD),
                                axis=AX)
                        q3 = qsb.rearrange("p (g d) -> p g d", d=HD)
                        sums = s2.tile([P, NG], f32, tag="hsum")
                        nc.vector.reduce_sum(sums, q3, axis=AX)
                        mean = s2.tile([P, NG], f32, tag="hmean")
                        nc.vector.tensor_scalar_mul(mean, sums, 1.0 / HD)
                        t2 = s2.tile([P, NG], f32, tag="ht2")
                        nc.vector.tensor_tensor(t2, sums, mean, ALU.mult)
                        var64 = s2.tile([P, NG], f32, tag="hvar")
                        nc.vector.tensor_tensor(var64, sumsq, t2, ALU.subtract)
                        std = s2.tile([P, NG], f32, tag="hstd")
                        nc.scalar.activation(std, var64, AF.Sqrt, bias=epsC,
                                             scale=1.0 / HD)
                        rinv = s2.tile([P, NG], f32, tag="hrinv")
                        nc.vector.reciprocal(rinv, std)
                        nc.vector.tensor_tensor(
                            q3, q3, mean[:, :, None].to_broadcast([P, NG, HD]),
                            ALU.subtract)
                        nc.vector.tensor_tensor(
                            q3, q3, rinv[:, :, None].to_broadcast([P, NG, HD]),
                            ALU.mult)
                        if hln_aff:
                            nc.vector.tensor_tensor(
                                q3, q3,
                                hlnw_sb[:, None, :].to_broadcast([P, NG, HD]),
                                ALU.mult)
                            nc.vector.tensor_tensor(
                                q3, q3,
                                hlnb_sb[:, None, :].to_broadcast([P, NG, HD]),
                                ALU.add)
                        zb = s2.tile([P, S3], bf16, tag="zb")
                        nc.vector.tensor_copy(zb, qsb)
                        for t in range(12):
                            transpose128(
                                zb[:, t * P:(t + 1) * P],
                                dst_dram_ap=qkT_t[s][t * P:(t + 1) * P,
                                                     c * P:(c + 1) * P])
                        nc.sync.dma_start(v_t[s][c * P:(c + 1) * P, :],
                                          zb[:, 2 * C:])
                        nc.sync.dma_start(
                            q2d_view[c * P:(c + 1) * P, :, :],
                            qsb[:, :C].rearrange("p (g d) -> p g d", d=HD))

            # ============ P3: two cross attentions ============
            pw64 = s1b.tile([HD, H, C], bf16, tag="w36")
            nc.sync.dma_start(pw64, proj_wT[:].rearrange("(h d) o -> d h o", d=HD))

            for (qs, ks) in (("b", "a"), ("a", "b")):
                # q from stream qs, k/v from ks, output added to stream ks
                with nc.named_scope(f"attn_{qs}{ks}"):
                    ctx = s1.tile([HD, H, N], bf16, name=f"ctx_{qs}", tag="big")
                    for h in range(H):
                        kT = s2.tile([HD, N], bf16, tag="kT")
                        nc.sync.dma_start(kT, qkT_t[ks][C + h * HD: C + (h + 1) * HD, :])
                        qT = s2.tile([HD, N], bf16, tag="qT")
                        nc.sync.dma_start(qT, qkT_t[qs][h * HD:(h + 1) * HD, :])
                        va = s2.tile([P, NCH, HD + 1], bf16, tag="vaug")
                        nc.vector.memset(va[:, :, HD:HD + 1], 1.0)
                        nc.sync.dma_start(
                            va[:, :, 0:HD],
                            v_t[ks][:].rearrange("(c p) f -> p c f", p=P)
                            [:, :, h * HD:(h + 1) * HD])
                        cpss = []
                        for nh in range(2):
                            cps = psB.tile([HD + 1, 512], f32, tag="psB",
                                           name=f"cps{nh}")
                            for mc in range(NCH):
                                sps = psA.tile([P, 512], f32, tag="psA")
                                nc.tensor.matmul(
                                    sps,
                                    kT[:, mc * P:(mc + 1) * P],
                                    qT[:, nh * 512:(nh + 1) * 512])
                                pt = s3.tile([P, 512], bf16, tag="pt")
                                nc.scalar.activation(pt, sps, AF.Exp,
                                                     scale=float(HD ** -0.5))
                                nc.tensor.matmul(cps, va[:, mc, :], pt,
                                                 start=(mc == 0), stop=(mc == NCH - 1))
                            cpss.append(cps)
                        for nh in range(2):
                            rec = s1.tile([P, 512], f32r, tag="recip")
                            with nc.allow_low_precision(reason="fp32r for matmul"):
                                nc.vector.reciprocal(rec[HD:HD + 1, :],
                                                     cpss[nh][HD:HD + 1, :])
                            bps = psC.tile([HD, 512], f32, tag="psC")
                            nc.tensor.matmul(bps, ones[HD:HD + 1, 0:HD],
                                             rec[HD:HD + 1, :])
                            bsb = s1b.tile([HD, 512], f32, tag="bsb")
                            nc.scalar.copy(bsb, bps)
                            nc.vector.tensor_tensor(
                                ctx[:, h, nh * 512:(nh + 1) * 512],
                                cpss[nh][0:HD, :], bsb, ALU.mult)

                    # proj + residual assembly + LN2 per chunk
                    q2dr = q2d_t[qs][:].rearrange("(n j) d -> n (j d)", j=H)
                    for c in range(NCH):
                        yps = []
                        for fh in range(2):
                            y = psB.tile([P, 384], f32, tag="psB")
                            for kc in range(H):
                                nc.tensor.matmul(
                                    y,
                                    ctx[:, kc, c * P:(c + 1) * P],
                                    pw64[:, kc, fh * 384:(fh + 1) * 384],
                                    start=(kc == 0),
                                    stop=(kc == H - 1 and not has_projb))
                            if has_projb:
                                nc.tensor.matmul(
                                    y, ones_bf[0:1, :],
                                    projb_sb[0:1, fh * 384:(fh + 1) * 384],
                                    start=False, stop=True)
                            yps.append(y)
                        xnr = s2.tile([P, C], f32, tag="xnres")
                        nc.sync.dma_start(xnr, xn_t[ks][c * P:(c + 1) * P, :])
                        qres = s2.tile([P, C], f32, tag="qres")
                        nc.sync.dma_start(qres, q2dr[c * P:(c + 1) * P, :])
                        ot = s2.tile([P, C], f32, tag="oassm")
                        for fh in range(2):
                            nc.vector.tensor_tensor(
                                ot[:, fh * 384:(fh + 1) * 384], yps[fh],
                                xnr[:, fh * 384:(fh + 1) * 384], ALU.add)
                        nc.vector.tensor_tensor(ot, ot, qres, ALU.add)
                        nc.sync.dma_start(o_t[ks][c * P:(c + 1) * P, :], ot)
                        x2 = s2.tile([P, C], f32, tag="lnout")
                        layernorm_chunk(ot, x2,
                                        n2w_sb if n2_aff else None,
                                        n2b_sb if n2_aff else None)
                        x2b = s2.tile([P, C], bf16, tag="xnb")
                        nc.vector.tensor_copy(x2b, x2)
                        for t in range(KC):
                            transpose128(
                                x2b[:, t * P:(t + 1) * P],
                                dst_dram_ap=x2T_t[ks][t * P:(t + 1) * P,
                                                      c * P:(c + 1) * P])

            # ============ P4: MLP per stream ============
            for s in "ab":
                with nc.named_scope(f"mlp_{s}"):
                    w1_view = fc1_wT[:].rearrange("(k p) f -> p k f", p=P)
                    w2_view = fc2_wT[:].rearrange("(k p) f -> p k f", p=P)
                    for nh in range(2):
                        x2h = s2.tile([P, KC, 512], bf16, tag="x2h")
                        nc.sync.dma_start(
                            x2h,
                            x2T_t[s][:].rearrange("(k p) n -> p k n", p=P)
                            [:, :, nh * 512:(nh + 1) * 512])
                        hT = s1.tile([P, HKC, 512], bf16, name=f"hT_{s}{nh}", tag="wstream")
                        for kc in range(HKC):
                            w1 = s3.tile([P, KC, P], bf16, tag="fc1w")
                            nc.sync.dma_start(w1, w1_view[:, :, kc * P:(kc + 1) * P])
                            fps = psA.tile([P, 512], f32, tag="psA")
                            for k in range(KC):
                                nc.tensor.matmul(
                                    fps, w1[:, k, :],
                                    x2h[:, k, :],
                                    start=(k == 0), stop=(k == KC - 1))
                            nc.scalar.activation(
                                hT[:, kc, :], fps, AF.Gelu,
                                bias=fc1b_sb[:, kc:kc + 1] if has_fc1b else 0.0)
                        for fh in range(2):
                            w2 = s1b.tile([P, HKC, 384], bf16,
                                          name=f"w2_{s}{nh}{fh}", tag="w36")
                            nc.sync.dma_start(
                                w2, w2_view[:, :, fh * 384:(fh + 1) * 384])
                            for sub in range(4):
                                c = nh * 4 + sub
                                y = psB.tile([P, 384], f32, tag="psB")
                                for kc in range(HKC):
                                    nc.tensor.matmul(
                                        y,
                                        hT[:, kc, sub * P:(sub + 1) * P],
                                        w2[:, kc, :],
                                        start=(kc == 0),
                                        stop=(kc == HKC - 1 and not has_fc2b))
                                if has_fc2b:
                                    nc.tensor.matmul(
                                        y, ones_bf[0:1, :],
                                        fc2b_sb[0:1, fh * 384:(fh + 1) * 384],
                                        start=False, stop=True)
                                oh = s2.tile([P, 384], f32, tag="ohalf")
                                nc.sync.dma_start(
                                    oh, o_t[s][c * P:(c + 1) * P,
                                               fh * 384:(fh + 1) * 384])
                                outt = s2.tile([P, 384], f32, tag="outc")
                                nc.vector.tensor_tensor(outt, y, oh, ALU.add)
                                nc.sync.dma_start(
                                    out_d[s][c * P:(c + 1) * P,
                                             fh * 384:(fh + 1) * 384], outt)

    nc.finalize()
    return nc


def _get_nc(flags):
    if flags not in _CACHE:
        _CACHE[flags] = _build(flags)
    return _CACHE[flags]


def _prep(inputs):
    import ml_dtypes

    f = np.float32
    bf = ml_dtypes.bfloat16
    w = {k: np.asarray(v, f) for k, v in inputs.items()}
    flags = (
        not (np.all(w["norm1_w"] == 1) and np.all(w["norm1_b"] == 0)),
        not (np.all(w["hln_w"] == 1) and np.all(w["hln_b"] == 0)),
        not (np.all(w["norm2_w"] == 1) and np.all(w["norm2_b"] == 0)),
        bool(np.any(w["proj_b"] != 0)),
        bool(np.any(w["fc1_b"] != 0)),
        bool(np.any(w["fc2_b"] != 0)),
    )
    shared = {
        "ones_in": np.ones(128, np.float32),
        "qkv_wT": np.ascontiguousarray(w["qkv_w"].T).astype(bf),
        "proj_wT": np.ascontiguousarray(w["proj_w"].T).astype(bf),
        "fc1_wT": np.ascontiguousarray(w["fc1_w"].T).astype(bf),
        "fc2_wT": np.ascontiguousarray(w["fc2_w"].T).astype(bf),
    }
    n1_aff, hln_aff, n2_aff, pb, f1b, f2b = flags
    if pb:
        shared["proj_b"] = w["proj_b"].reshape(1, C).astype(bf)
    if f1b:
        shared["fc1_b"] = w["fc1_b"]
    if f2b:
        shared["fc2_b"] = w["fc2_b"].reshape(1, C).astype(bf)
    if n1_aff:
        shared["norm1_w"] = w["norm1_w"]
        shared["norm1_b"] = w["norm1_b"]
    if n2_aff:
        shared["norm2_w"] = w["norm2_w"]
        shared["norm2_b"] = w["norm2_b"]
    if hln_aff:
        shared["hln_w"] = w["hln_w"]
        shared["hln_b"] = w["hln_b"]
    return w, flags, shared


def kernel(trace=False, **inputs):
    from concourse.bass_utils import run_bass_kernel_spmd

    w, flags, shared = _prep(inputs)
    nc = _get_nc(flags)
    before = np.ascontiguousarray(w["before"], dtype=np.float32)
    after = np.ascontiguousarray(w["after"], dtype=np.float32)
    in_maps = []
    for core in range(B):
        m = dict(shared)
        m["x_b"] = np.ascontiguousarray(before[core])
        m["x_a"] = np.ascontiguousarray(after[core])
        in_maps.append(m)
    res = run_bass_kernel_spmd(nc, in_maps, core_ids=list(range(B)), trace=trace)
    before_o = np.stack([res.results[i]["out_b"] for i in range(B)])
    after_o = np.stack([res.results[i]["out_a"] for i in range(B)])
    out = (before_o.astype(np.float32), after_o.astype(np.float32))
    if trace:
        return out, res
    return out



# revision 10
# speedup vs baseline: 1.2926x; 1.2926x over previous
"""Trainium2 Bass kernel for nn_Block_22325240004804 (dense_transformer).

Two-stream cross-attention transformer block, B=8 N=1024 C=768 H=12.
Sharding: pure data parallel - batch element b on core b (no collectives).

v4 redesign (from v3 trace analysis: PE 68% busy with bursty waits, HAM
cold 87% of time, Vector gated QKV via head-LN chain, single-lane
reciprocals, 647 DMA issues on sync):
  - head-LN centering folded into qkv weights on host (W' = W - groupmean(W));
    variance is then just sumsq/64: kills the mean/sub vector chain.
  - PV stationary packs [v | ones] (ones block on the opposite half per head
    parity), so the softmax denominator comes out REPLICATED across 64
    partitions of the PV output: reciprocal_approx_fast in place, one
    partition-shifting SBUF DMA, one lane-aligned TT mult normalizes a head.
  - proj contracts head PAIRS (ctx2 [128, 6, N]) -> full 128-row PE util.
  - exp batched [128, 1024] over 2 PSUM banks; S/exp/PV pipelined via 4x
    2-bank PSUM slots shared by all phases.
  - streams and attention directions interleaved for engine overlap;
    loads on sync queue, stores on gpsimd queue, stores consolidated.
  - SBUF: one 24KB "big" tag rotates xnT -> ctx2 -> hT lifetimes; one
    rotating f32 chunk-buffer tag; fc2 weights share the qkv weight slot.
"""

import sys

if "/opt/trn_rl_repo" not in sys.path:
    sys.path.insert(0, "/opt/trn_rl_repo")

import numpy as np

B, N, C = 8, 1024, 768
H, HD = 12, 64
S3 = 3 * C          # 2304
HID = 4 * C         # 3072
EPS = 1e-5
P = 128
NCH = N // P        # 8 token chunks
KC = C // P         # 6 contraction chunks over C
NG = S3 // HD       # 36 head-groups per token row
HKC = HID // P      # 24 chunks over HID
NJ = H // 2         # 6 head pairs

_CACHE = {}


def _build(flags):
    import concourse.bass as bass
    import concourse.tile as tile
    from concourse import bacc, mybir

    f32 = mybir.dt.float32
    bf16 = mybir.dt.bfloat16
    AF = mybir.ActivationFunctionType
    ALU = mybir.AluOpType
    AX = mybir.AxisListType.X

    (n1_aff, hln_aff, n2_aff, has_projb, has_fc1b, has_fc2b) = flags

    nc = bacc.Bacc("TRN2", target_bir_lowering=False)

    # ---------------- I/O ----------------
    x_in = {
        "b": nc.dram_tensor("x_b", [N, C], f32, kind="ExternalInput"),
        "a": nc.dram_tensor("x_a", [N, C], f32, kind="ExternalInput"),
    }
    qkv_wT = nc.dram_tensor("qkv_wT", [C, S3], bf16, kind="ExternalInput")
    pw2_d = nc.dram_tensor("pw2", [P, NJ * C], bf16, kind="ExternalInput")
    w1p_d = nc.dram_tensor("w1p", [P, HKC * C], bf16, kind="ExternalInput")
    w2p_d = nc.dram_tensor("w2p", [P, HKC * C], bf16, kind="ExternalInput")
    projb_d = nc.dram_tensor("proj_b", [1, C], bf16, kind="ExternalInput") if has_projb else None
    fc1b_d = nc.dram_tensor("fc1_b", [HID], f32, kind="ExternalInput") if has_fc1b else None
    fc2b_d = nc.dram_tensor("fc2_b", [1, C], bf16, kind="ExternalInput") if has_fc2b else None
    n1w_d = nc.dram_tensor("norm1_w", [C], f32, kind="ExternalInput") if n1_aff else None
    n1b_d = nc.dram_tensor("norm1_b", [C], f32, kind="ExternalInput") if n1_aff else None
    n2w_d = nc.dram_tensor("norm2_w", [C], f32, kind="ExternalInput") if n2_aff else None
    n2b_d = nc.dram_tensor("norm2_b", [C], f32, kind="ExternalInput") if n2_aff else None
    hlnw_d = nc.dram_tensor("hln_w", [HD], f32, kind="ExternalInput") if hln_aff else None
    hlnb_d = nc.dram_tensor("hln_b", [HD], f32, kind="ExternalInput") if hln_aff else None
    out_d = {
        "b": nc.dram_tensor("out_b", [N, C], f32, kind="ExternalOutput"),
        "a": nc.dram_tensor("out_a", [N, C], f32, kind="ExternalOutput"),
    }

    with tile.TileContext(nc) as tc:
        with (
            tc.tile_pool(name="dram", bufs=1, space="DRAM") as dram,
            tc.tile_pool(name="const", bufs=1) as const,
            tc.tile_pool(name="big", bufs=3) as big,    # xnT/ctx2/hT rotate
            tc.tile_pool(name="s1", bufs=1) as s1,      # weights
            tc.tile_pool(name="sB", bufs=2) as sB,
            tc.tile_pool(name="s2", bufs=2) as s2,
            tc.tile_pool(name="sF", bufs=4) as sF,      # f32 chunk buffers
            tc.tile_pool(name="s3", bufs=2) as s3,
            tc.tile_pool(name="ps", bufs=4, space="PSUM") as ps,
        ):
            # -------- DRAM staging --------
            xn_t = {s: dram.tile([N, C], f32, name=f"xn_{s}", tag=f"xn_{s}") for s in "ba"}
            qkT_t = {s: dram.tile([2 * C, N], bf16, name=f"qkT_{s}", tag=f"qkT_{s}") for s in "ba"}
            v_t = {s: dram.tile([N, C], bf16, name=f"v_{s}", tag=f"v_{s}") for s in "ba"}
            qr_t = {s: dram.tile([H * N, HD], bf16, name=f"qr_{s}", tag=f"qr_{s}") for s in "ba"}
            o_t = {s: dram.tile([N, C], f32, name=f"o_{s}", tag=f"o_{s}") for s in "ba"}
            x2T_t = {s: dram.tile([C, N], bf16, name=f"x2T_{s}", tag=f"x2T_{s}") for s in "ba"}

            # -------- constants --------
            from concourse.masks import make_identity
            ident = const.tile([P, P], bf16, tag="ident")
            make_identity(nc, ident)
            epsC = const.tile([P, 1], f32, tag="epsC")
            nc.vector.memset(epsC, EPS)

            if has_projb or has_fc2b:
                ones_bf = const.tile([1, P], bf16, tag="ones_bf")
                nc.vector.memset(ones_bf, 1.0)
            if has_projb:
                projb_sb = const.tile([1, C], bf16, tag="projb")
                nc.sync.dma_start(projb_sb, projb_d[:])
            if has_fc2b:
                fc2b_sb = const.tile([1, C], bf16, tag="fc2b")
                nc.sync.dma_start(fc2b_sb, fc2b_d[:])
            if has_fc1b:
                fc1b_sb = const.tile([P, HKC], f32, tag="fc1b")
                nc.sync.dma_start(fc1b_sb, fc1b_d[:].rearrange("(k p) -> p k", p=P))

            def bcast_load(src_ap, cols, tag):
                t = const.tile([P, cols], f32, tag=tag)
                bc = bass.AP(tensor=src_ap.tensor, offset=src_ap.offset,
                             ap=[[0, P]] + list(src_ap.ap))
                nc.gpsimd.dma_start(out=t, in_=bc)
                return t

            if n1_aff:
                n1w_sb = bcast_load(n1w_d[:], C, "n1w")
                n1b_sb = bcast_load(n1b_d[:], C, "n1b")
            if n2_aff:
                n2w_sb = bcast_load(n2w_d[:], C, "n2w")
                n2b_sb = bcast_load(n2b_d[:], C, "n2b")
            if hln_aff:
                hlnw_sb = bcast_load(hlnw_d[:], HD, "hlnw")
                hlnb_sb = bcast_load(hlnb_d[:], HD, "hlnb")

            # -------- helpers --------
            def layernorm_chunk(x_tile, out_tile, w_sb, b_sb):
                """LN over free dim 768 of a [128, 768] tile (fp32)."""
                st = s2.tile([P, 3, 6], f32, tag="lnst")
                for g in range(3):
                    nc.vector.bn_stats(st[:, g, :], x_tile[:, g * 256:(g + 1) * 256])
                mv = s2.tile([P, 2], f32, tag="lnmv")
                nc.vector.bn_aggr(mv, st)
                std = s2.tile([P, 1], f32, tag="lnstd")
                nc.scalar.activation(std, mv[:, 1:2], AF.Sqrt, bias=epsC)
                rstd = s2.tile([P, 1], f32, tag="lnrstd")
                nc.vector.reciprocal(rstd, std)
                nc.vector.tensor_scalar(out_tile, x_tile, mv[:, 0:1], rstd,
                                        ALU.subtract, ALU.mult)
                if w_sb is not None:
                    nc.vector.tensor_tensor(out_tile, out_tile, w_sb, ALU.mult)
                    nc.vector.tensor_tensor(out_tile, out_tile, b_sb, ALU.add)

            # ======== P1 + QKV, streams interleaved per chunk ========
            xnTd = {}
            for s in "ba":
                xnTd[s] = big.tile([P, HKC, 512], bf16, name=f"xnT_{s}", tag="big")
            wq = s1.tile([P, KC, S3], bf16, tag="wbig")
            nc.sync.dma_start(wq, qkv_wT[:].rearrange("(k p) f -> p k f", p=P))
            pw2sb = s1.tile([P, NJ, C], bf16, tag="pw2sb")
            nc.sync.dma_start(pw2sb, pw2_d[:].rearrange("p (j o) -> p j o", o=C))

            def xnT_ap(s, k, cslice):
                # logical [P, KC, N] view on the [P, HKC, 512] big-tag tile
                return xnTd[s].rearrange("p (k x) n -> p k (x n)", x=2)[:, k, cslice]

            QSL = [(0, 1024), (1024, 1024), (2048, 256)]

            for c in range(NCH):
                cs = slice(c * P, (c + 1) * P)
                for s in "ba":
                    with nc.named_scope(f"p1_{s}"):
                        xt = sF.tile([P, C], f32, tag="f32buf", name="xt")
                        nc.sync.dma_start(xt, x_in[s][cs, :])
                        xn = sF.tile([P, C], f32, tag="f32buf", name="xn")
                        layernorm_chunk(xt, xn,
                                        n1w_sb if n1_aff else None,
                                        n1b_sb if n1_aff else None)
                        nc.gpsimd.dma_start(xn_t[s][cs, :], xn)
                        xnb = s2.tile([P, C], bf16, tag="xnb")
                        nc.gpsimd.tensor_copy(xnb, xn)
                        tp = ps.tile([P, KC, P], bf16, tag="A", name="tp1")
                        for t in range(KC):
                            nc.tensor.transpose(tp[:, t, :], xnb[:, t * P:(t + 1) * P], ident)
                        for t in range(KC):
                            nc.vector.tensor_copy(xnT_ap(s, t, cs), tp[:, t, :])

                for s in "ba":
                    with nc.named_scope(f"qkv_{s}"):
                        accs = []
                        for i, (f0, fw) in enumerate(QSL):
                            acc = ps.tile([P, fw], f32, tag="A", name=f"qacc{i}")
                            for k in range(KC):
                                for m0 in range(0, fw, 512):
                                    mw = min(512, fw - m0)
                                    nc.tensor.matmul(
                                        acc[:, m0:m0 + mw],
                                        xnT_ap(s, k, cs),
                                        wq[:, k, f0 + m0:f0 + m0 + mw],
                                        start=(k == 0), stop=(k == KC - 1))
                            accs.append(acc)
                        # squares -> sumsq -> rstd  (centering folded into W')
                        sumsq = s2.tile([P, NG], f32, tag="hsumsq")
                        for i, (f0, fw) in enumerate(QSL):
                            sq = s2.tile([P, 1024], bf16, tag="sq")
                            nc.scalar.activation(sq[:, :fw], accs[i], AF.Square)
                            nc.vector.reduce_sum(
                                sumsq[:, f0 // HD:(f0 + fw) // HD],
                                sq[:, :fw].rearrange("p (g d) -> p g d", d=HD),
                                axis=AX)
                        stdq = s2.tile([P, NG], f32, tag="hstd")
                        nc.scalar.activation(stdq, sumsq, AF.Sqrt, bias=epsC,
                                             scale=1.0 / HD)
                        rstd = s2.tile([P, NG], f32, tag="hrstd")
                        nc.vector.reciprocal(rstd, stdq)
                        # normalize (+ optional affine) straight out of PSUM -> bf16
                        zb = s2.tile([P, S3], bf16, tag="zb")
                        for i, (f0, fw) in enumerate(QSL):
                            g0 = f0 // HD
                            gw = fw // HD
                            zv = zb[:, f0:f0 + fw].rearrange("p (g d) -> p g d", d=HD)
                            nc.vector.tensor_tensor(
                                zv,
                                accs[i].rearrange("p (g d) -> p g d", d=HD),
                                rstd[:, g0:g0 + gw, None].to_broadcast([P, gw, HD]),
                                ALU.mult)
                        if hln_aff:
                            z3 = zb.rearrange("p (g d) -> p g d", d=HD)
                            nc.vector.tensor_tensor(
                                z3, z3, hlnw_sb[:, None, :].to_broadcast([P, NG, HD]),
                                ALU.mult)
                            nc.vector.tensor_tensor(
                                z3, z3, hlnb_sb[:, None, :].to_broadcast([P, NG, HD]),
                                ALU.add)
                        # transposes of q,k sections (12 x 128)
                        for half in range(2):
                            tp2 = ps.tile([P, KC, P], bf16, tag="A", name="tp2")
                            for t in range(KC):
                                tt = half * KC + t
                                nc.tensor.transpose(
                                    tp2[:, t, :], zb[:, tt * P:(tt + 1) * P], ident)
                            qkt_sb = s2.tile([P, KC, P], bf16, tag="qkt")
                            nc.vector.tensor_copy(qkt_sb, tp2)
                            nc.gpsimd.dma_start(
                                qkT_t[s][:].rearrange("(t p) n -> p t n", p=P)
                                [:, half * KC:(half + 1) * KC, cs],
                                qkt_sb)
                        nc.gpsimd.dma_start(v_t[s][cs, :], zb[:, 2 * C:])
                        nc.gpsimd.dma_start(
                            qr_t[s][:].rearrange("(h n) d -> n h d", h=H)[cs],
                            zb[:, :C].rearrange("p (g d) -> p g d", d=HD))

            # ======== Attention: both directions, head-pair interleaved ========
            DIRS = (("b", "a"), ("a", "b"))  # (qs, ks); output goes to stream ks
            ctx2 = {}
            for qs, ks in DIRS:
                ctx2[qs] = big.tile([P, HKC, 512], bf16, name=f"ctx2_{qs}", tag="big")

            def ctx2_ap(qs, jj, cslice):
                return ctx2[qs].rearrange("p (j x) n -> p j (x n)", x=2)[:, jj, cslice]

            for j in range(NJ):
                for hp in range(2):
                    h = 2 * j + hp
                    hs = slice(hp * HD, (hp + 1) * HD)        # ctx half
                    ds = slice((1 - hp) * HD, (2 - hp) * HD)  # denominator half
                    lo = slice(0, HD)
                    for qs, ks in DIRS:
                        with nc.named_scope(f"attn_{qs}"):
                            # per-head k/q tiles at partition base 0 (offset
                            # matmuls crash HW; custom-DVE ops break at
                            # partition offsets -- keep compute at base 0)
                            qt = s3.tile([HD, N], bf16, tag="qh",
                                         name=f"qh_{qs}{h}")
                            nc.sync.dma_start(
                                qt, qkT_t[qs][h * HD:(h + 1) * HD, :])
                            kt = s3.tile([HD, N], bf16, tag="kh",
                                         name=f"kh_{qs}{h}")
                            nc.sync.dma_start(
                                kt, qkT_t[ks][C + h * HD:C + (h + 1) * HD, :])
                            va = s3.tile([P, NCH, P], bf16, tag="va")
                            nc.gpsimd.memset(
                                va[:, :, (1 - hp) * HD:(2 - hp) * HD], 1.0)
                            nc.sync.dma_start(
                                va[:, :, hp * HD:(hp + 1) * HD],
                                v_t[ks][:].rearrange("(c p) f -> p c f", p=P)
                                [:, :, h * HD:(h + 1) * HD])
                            cps = ps.tile([P, 2, 512], f32, tag="A", name="cps")
                            for mc in range(NCH):
                                sps = ps.tile([P, 2, 512], f32, tag="A", name="sps")
                                for nh in range(2):
                                    nc.tensor.matmul(
                                        sps[:, nh, :],
                                        kt[:, mc * P:(mc + 1) * P],
                                        qt[:, nh * 512:(nh + 1) * 512])
                                pt = s3.tile([P, 2, 512], bf16, tag="pt")
                                nc.scalar.activation(
                                    pt.rearrange("p a b -> p (a b)"),
                                    sps.rearrange("p a b -> p (a b)"),
                                    AF.Exp, scale=float(HD ** -0.5))
                                for nh in range(2):
                                    nc.tensor.matmul(
                                        cps[:, nh, :], va[:, mc, :], pt[:, nh, :],
                                        start=(mc == 0), stop=(mc == NCH - 1))
                            # denominator (replicated on partitions ds):
                            # lane-aligned copy out of PSUM, shift to base 0,
                            # recipfast at base 0, shift result to hs, mult.
                            dn = s3.tile([P, N], f32, tag="dn")
                            nc.scalar.copy(
                                dn[ds, :],
                                cps[ds, :, :].rearrange("p a b -> p (a b)"))
                            if hp == 0:
                                nc.sync.dma_start(dn[lo, :], dn[ds, :])
                            rd = s3.tile([P, N], f32, tag="rd")
                            nc.vector.reciprocal_approx_fast(rd[lo, :], dn[lo, :])
                            if hp == 1:
                                nc.sync.dma_start(rd[hs, :], rd[lo, :])
                            nc.vector.tensor_tensor(
                                ctx2_ap(qs, j, slice(0, N))[hs, :],
                                cps[hs, :, :].rearrange("p a b -> p (a b)"),
                                rd[hs, :], ALU.mult)

            # ======== proj + residual + LN2 per direction ========
            for qs, ks in DIRS:
                with nc.named_scope(f"proj_{ks}"):
                    qr_view = qr_t[qs][:].rearrange("(n j) d -> n (j d)", j=H)
                    for c in range(NCH):
                        cs = slice(c * P, (c + 1) * P)
                        y = ps.tile([P, C], f32, tag="A", name="yproj")
                        for jj in range(NJ):
                            for o0, ow in ((0, 512), (512, 256)):
                                nc.tensor.matmul(
                                    y[:, o0:o0 + ow],
                                    ctx2_ap(qs, jj, cs),
                                    pw2sb[:, jj, o0:o0 + ow],
                                    start=(jj == 0),
                                    stop=(jj == NJ - 1 and not has_projb))
                        if has_projb:
                            for o0, ow in ((0, 512), (512, 256)):
                                nc.tensor.matmul(
                                    y[:, o0:o0 + ow], ones_bf[0:1, :],
                                    projb_sb[0:1, o0:o0 + ow],
                                    start=False, stop=True)
                        xnr = sF.tile([P, C], f32, tag="f32buf", name="xnr")
                        nc.sync.dma_start(xnr, xn_t[ks][cs, :])
                        qres = s2.tile([P, C], bf16, tag="qres")
                        nc.sync.dma_start(qres, qr_view[cs, :])
                        ot = sF.tile([P, C], f32, tag="f32buf", name="ot")
                        nc.vector.tensor_tensor(ot, y, xnr, ALU.add)
                        nc.vector.tensor_tensor(ot, ot, qres, ALU.add)
                        nc.gpsimd.dma_start(o_t[ks][cs, :], ot)
                        x2 = sF.tile([P, C], f32, tag="f32buf", name="x2")
                        layernorm_chunk(ot, x2,
                                        n2w_sb if n2_aff else None,
                                        n2b_sb if n2_aff else None)
                        x2b = s2.tile([P, C], bf16, tag="xnb")
                        nc.gpsimd.tensor_copy(x2b, x2)
                        tp3 = ps.tile([P, KC, P], bf16, tag="A", name="tp3")
                        for t in range(KC):
                            nc.tensor.transpose(tp3[:, t, :], x2b[:, t * P:(t + 1) * P],
                                                ident)
                        x2ts = s2.tile([P, KC, P], bf16, tag="x2ts")
                        nc.vector.tensor_copy(x2ts, tp3)
                        nc.gpsimd.dma_start(
                            x2T_t[ks][:].rearrange("(t p) n -> p t n", p=P)[:, :, cs],
                            x2ts)

            # ======== MLP per stream (a first: its residual is ready first) ====
            w2sb = s1.tile([P, HKC, C], bf16, tag="wbig")
            nc.sync.dma_start(w2sb, w2p_d[:].rearrange("p (k o) -> p k o", o=C))
            w1v = w1p_d[:].rearrange("p (kc k f) -> p kc k f", k=KC, f=P)
            for s in "ab":
                with nc.named_scope(f"mlp_{s}"):
                    x2h = []
                    for nh in range(2):
                        xh = sB.tile([P, KC, 512], bf16, tag="x2h", name=f"x2h_{s}{nh}")
                        nc.sync.dma_start(
                            xh,
                            x2T_t[s][:].rearrange("(k p) n -> p k n", p=P)
                            [:, :, nh * 512:(nh + 1) * 512])
                        x2h.append(xh)
                    hT = []
                    for nh in range(2):
                        hT.append(big.tile([P, HKC, 512], bf16, tag="big",
                                           name=f"hT_{s}{nh}"))
                    for kc2 in range(HKC // 2):
                        w1k = []
                        for kk in range(2):
                            w1t = s3.tile([P, KC, P], bf16, tag="w1k")
                            nc.scalar.dma_start(w1t, w1v[:, 2 * kc2 + kk, :, :])
                            w1k.append(w1t)
                        accs = [ps.tile([P, 2, 512], f32, tag="A", name=f"facc{nh}")
                                for nh in range(2)]
                        for k in range(KC):
                            for kk in range(2):
                                for nh in range(2):
                                    nc.tensor.matmul(
                                        accs[nh][:, kk, :],
                                        w1k[kk][:, k, :],
                                        x2h[nh][:, k, :],
                                        start=(k == 0), stop=(k == KC - 1))
                        for nh in range(2):
                            if has_fc1b:
                                for kk in range(2):
                                    kc = 2 * kc2 + kk
                                    nc.scalar.activation(
                                        hT[nh][:, kc, :], accs[nh][:, kk, :],
                                        AF.Gelu, bias=fc1b_sb[:, kc:kc + 1])
                            else:
                                nc.scalar.activation(
                                    hT[nh][:, 2 * kc2:2 * kc2 + 2, :]
                                    .rearrange("p a b -> p (a b)"),
                                    accs[nh].rearrange("p a b -> p (a b)"),
                                    AF.Gelu)
                    for nh in range(2):
                        for sub in range(4):
                            c = nh * 4 + sub
                            cs = slice(c * P, (c + 1) * P)
                            y = ps.tile([P, C], f32, tag="A", name="yfc2")
                            for kc in range(HKC):
                                for o0, ow in ((0, 512), (512, 256)):
                                    nc.tensor.matmul(
                                        y[:, o0:o0 + ow],
                                        hT[nh][:, kc, sub * P:(sub + 1) * P],
                                        w2sb[:, kc, o0:o0 + ow],
                                        start=(kc == 0),
                                        stop=(kc == HKC - 1 and not has_fc2b))
                            if has_fc2b:
                                for o0, ow in ((0, 512), (512, 256)):
                                    nc.tensor.matmul(
                                        y[:, o0:o0 + ow], ones_bf[0:1, :],
                                        fc2b_sb[0:1, o0:o0 + ow],
                                        start=False, stop=True)
                            oh = sF.tile([P, C], f32, tag="f32buf", name="oh")
                            nc.sync.dma_start(oh, o_t[s][cs, :])
                            outt = sF.tile([P, C], f32, tag="f32buf", name="outt")
                            nc.vector.tensor_tensor(outt, y, oh, ALU.add)
                            nc.gpsimd.dma_start(out_d[s][cs, :], outt)

    nc.finalize()
    return nc


def _get_nc(flags):
    if flags not in _CACHE:
        _CACHE[flags] = _build(flags)
    return _CACHE[flags]


def _prep(inputs):
    import ml_dtypes

    f = np.float32
    bf = ml_dtypes.bfloat16
    w = {k: np.asarray(v, f) for k, v in inputs.items()}
    flags = (
        not (np.all(w["norm1_w"] == 1) and np.all(w["norm1_b"] == 0)),
        not (np.all(w["hln_w"] == 1) and np.all(w["hln_b"] == 0)),
        not (np.all(w["norm2_w"] == 1) and np.all(w["norm2_b"] == 0)),
        bool(np.any(w["proj_b"] != 0)),
        bool(np.any(w["fc1_b"] != 0)),
        bool(np.any(w["fc2_b"] != 0)),
    )
    # qkv weights: transpose + fold head-LN centering (linear in x)
    wT = np.ascontiguousarray(w["qkv_w"].T)                   # [C, 3C]
    wT3 = wT.reshape(C, NG, HD)
    wTc = (wT3 - wT3.mean(axis=2, keepdims=True)).reshape(C, S3)
    # proj weights packed by head pair: pw2[p=(h%2)*64+d, j=h//2, o]
    pw = w["proj_w"].T.reshape(NJ, 2, HD, C).transpose(1, 2, 0, 3).reshape(P, NJ * C)
    # fc1 packed: w1p[p, kc, k, f'] = fc1_w[kc*128+f', k*128+p]
    w1p = w["fc1_w"].reshape(HKC, P, KC, P).transpose(3, 0, 2, 1).reshape(P, HKC * C)
    # fc2 packed: w2p[p, kc, o] = fc2_w[o, kc*128+p]
    w2p = w["fc2_w"].reshape(C, HKC, P).transpose(2, 1, 0).reshape(P, HKC * C)
    shared = {
        "qkv_wT": wTc.astype(bf),
        "pw2": np.ascontiguousarray(pw).astype(bf),
        "w1p": np.ascontiguousarray(w1p).astype(bf),
        "w2p": np.ascontiguousarray(w2p).astype(bf),
    }
    n1_aff, hln_aff, n2_aff, pb, f1b, f2b = flags
    if pb:
        shared["proj_b"] = w["proj_b"].reshape(1, C).astype(bf)
    if f1b:
        shared["fc1_b"] = w["fc1_b"]
    if f2b:
        shared["fc2_b"] = w["fc2_b"].reshape(1, C).astype(bf)
    if n1_aff:
        shared["norm1_w"] = w["norm1_w"]
        shared["norm1_b"] = w["norm1_b"]
    if n2_aff:
        shared["norm2_w"] = w["norm2_w"]
        shared["norm2_b"] = w["norm2_b"]
    if hln_aff:
        shared["hln_w"] = w["hln_w"]
        shared["hln_b"] = w["hln_b"]
    return w, flags, shared


def kernel(trace=False, **inputs):
    from concourse.bass_utils import run_bass_kernel_spmd

    w, flags, shared = _prep(inputs)
    nc = _get_nc(flags)
    before = np.ascontiguousarray(w["before"], dtype=np.float32)
    after = np.ascontiguousarray(w["after"], dtype=np.float32)
    in_maps = []
    for core in range(B):
        m = dict(shared)
        m["x_b"] = np.ascontiguousarray(before[core])
        m["x_a"] = np.ascontiguousarray(after[core])
        in_maps.append(m)
    res = run_bass_kernel_spmd(nc, in_maps, core_ids=list(range(B)), trace=trace)
    before_o = np.stack([res.results[i]["out_b"] for i in range(B)])
    after_o = np.stack([res.results[i]["out_a"] for i in range(B)])
    out = (before_o.astype(np.float32), after_o.astype(np.float32))
    if trace:
        return out, res
    return out


# revision 13
# speedup vs baseline: 1.7285x; 1.3373x over previous
"""Trainium2 Bass kernel for nn_Block_22325240004804 (dense_transformer).

Two-stream cross-attention transformer block, B=8 N=1024 C=768 H=12.
Sharding: pure data parallel - batch element b on core b (no collectives).

v5 (on top of v4.1):
  - head-LN centering folded into qkv weights on host (W' = W - groupmean(W))
  - PV stationary packs [v | ones] per head parity -> softmax denominator
    replicated across 64 partitions; drain: lane-aligned PSUM copy, shift
    DMA to partition 0 (custom-DVE ops break at partition offsets on HW,
    offset matmuls crash), recipfast at base 0, shift back, aligned mult
  - proj contracts head PAIRS (ctx2 [128, 6, N]) -> full 128-row PE util
  - fp8e4 + DoubleRow for fc1/fc2 (weights x32 host-scaled; gelu scale
    1/32 folds it; fc2 output x(1/32) fused into residual add) and for the
    PV matmul (exp emits fp8 with a -ln16 bias shift; denominators scale
    identically so normalization is exact)
  - persistent va tiles (parity x direction) - no per-head ones memsets
  - DMA issue spread: loads+stores on sync, casting loads/stores on gpsimd,
    residual loads + denominator shifts on vector, fc1 weights on scalar
"""

import sys

if "/opt/trn_rl_repo" not in sys.path:
    sys.path.insert(0, "/opt/trn_rl_repo")

import numpy as np

B, N, C = 8, 1024, 768
H, HD = 12, 64
S3 = 3 * C          # 2304
HID = 4 * C         # 3072
EPS = 1e-5
P = 128
NCH = N // P        # 8 token chunks
KC = C // P         # 6 contraction chunks over C
NG = S3 // HD       # 36 head-groups per token row
HKC = HID // P      # 24 chunks over HID
NJ = H // 2         # 6 head pairs
W8SCALE = 32.0      # host scale on fp8 mlp weights
ELN16 = -2.772588722239781  # -ln(16): fp8-range shift for exp

_CACHE = {}


def _build(flags):
    import concourse.bass as bass
    import concourse.tile as tile
    from concourse import bacc, mybir

    f32 = mybir.dt.float32
    bf16 = mybir.dt.bfloat16
    f8 = mybir.dt.float8e4
    AF = mybir.ActivationFunctionType
    ALU = mybir.AluOpType
    AX = mybir.AxisListType.X
    DR = mybir.MatmulPerfMode.DoubleRow

    (n1_aff, hln_aff, n2_aff, has_projb, has_fc1b, has_fc2b) = flags

    nc = bacc.Bacc("TRN2", target_bir_lowering=False)

    # ---------------- I/O ----------------
    x_in = {
        "b": nc.dram_tensor("x_b", [N, C], f32, kind="ExternalInput"),
        "a": nc.dram_tensor("x_a", [N, C], f32, kind="ExternalInput"),
    }
    qkv_wT = nc.dram_tensor("qkv_wT", [C, S3], bf16, kind="ExternalInput")
    pw2_d = nc.dram_tensor("pw2", [P, NJ * C], bf16, kind="ExternalInput")
    w1p_d = nc.dram_tensor("w1p", [P, HKC * C], f8, kind="ExternalInput")
    w2p_d = nc.dram_tensor("w2p", [P, HKC * C], f8, kind="ExternalInput")
    projb_d = nc.dram_tensor("proj_b", [1, C], bf16, kind="ExternalInput") if has_projb else None
    fc1b_d = nc.dram_tensor("fc1_b", [HID], f32, kind="ExternalInput") if has_fc1b else None
    fc2b_d = nc.dram_tensor("fc2_b", [1, C], bf16, kind="ExternalInput") if has_fc2b else None
    n1w_d = nc.dram_tensor("norm1_w", [C], f32, kind="ExternalInput") if n1_aff else None
    n1b_d = nc.dram_tensor("norm1_b", [C], f32, kind="ExternalInput") if n1_aff else None
    n2w_d = nc.dram_tensor("norm2_w", [C], f32, kind="ExternalInput") if n2_aff else None
    n2b_d = nc.dram_tensor("norm2_b", [C], f32, kind="ExternalInput") if n2_aff else None
    hlnw_d = nc.dram_tensor("hln_w", [HD], f32, kind="ExternalInput") if hln_aff else None
    hlnb_d = nc.dram_tensor("hln_b", [HD], f32, kind="ExternalInput") if hln_aff else None
    out_d = {
        "b": nc.dram_tensor("out_b", [N, C], f32, kind="ExternalOutput"),
        "a": nc.dram_tensor("out_a", [N, C], f32, kind="ExternalOutput"),
    }

    with tile.TileContext(nc) as tc:
        with (
            tc.tile_pool(name="dram", bufs=1, space="DRAM") as dram,
            tc.tile_pool(name="const", bufs=1) as const,
            tc.tile_pool(name="big", bufs=3) as big,    # xnT/ctx2/hT rotate
            tc.tile_pool(name="s1", bufs=1) as s1,      # weights + va
            tc.tile_pool(name="sB", bufs=2) as sB,
            tc.tile_pool(name="s2", bufs=2) as s2,
            tc.tile_pool(name="sF", bufs=4) as sF,      # f32 chunk buffers
            tc.tile_pool(name="s3", bufs=2) as s3,
            tc.tile_pool(name="s3b", bufs=3) as s3b,
            tc.tile_pool(name="ps", bufs=4, space="PSUM") as ps,
        ):
            # -------- DRAM staging --------
            xn_t = {s: dram.tile([N, C], f32, name=f"xn_{s}", tag=f"xn_{s}") for s in "ba"}
            qkT_t = {s: dram.tile([2 * C, N], bf16, name=f"qkT_{s}", tag=f"qkT_{s}") for s in "ba"}
            v_t = {s: dram.tile([N, C], f8, name=f"v_{s}", tag=f"v_{s}") for s in "ba"}
            qr_t = {s: dram.tile([H * N, HD], bf16, name=f"qr_{s}", tag=f"qr_{s}") for s in "ba"}
            o_t = {s: dram.tile([N, C], f32, name=f"o_{s}", tag=f"o_{s}") for s in "ba"}
            x2T_t = {s: dram.tile([C, N], bf16, name=f"x2T_{s}", tag=f"x2T_{s}") for s in "ba"}

            # -------- constants --------
            from concourse.masks import make_identity
            ident = const.tile([P, P], bf16, tag="ident")
            make_identity(nc, ident)
            epsC = const.tile([P, 1], f32, tag="epsC")
            nc.vector.memset(epsC, EPS)
            eln = const.tile([P, 1], f32, tag="eln")
            nc.vector.memset(eln, ELN16)

            if has_projb or has_fc2b:
                ones_bf = const.tile([1, P], bf16, tag="ones_bf")
                nc.vector.memset(ones_bf, 1.0)
            if has_projb:
                projb_sb = const.tile([1, C], bf16, tag="projb")
                nc.sync.dma_start(projb_sb, projb_d[:])
            if has_fc2b:
                fc2b_sb = const.tile([1, C], bf16, tag="fc2b")
                nc.sync.dma_start(fc2b_sb, fc2b_d[:])
            if has_fc1b:
                fc1b_sb = const.tile([P, HKC], f32, tag="fc1b")
                nc.sync.dma_start(fc1b_sb, fc1b_d[:].rearrange("(k p) -> p k", p=P))

            def bcast_load(src_ap, cols, tag):
                t = const.tile([P, cols], f32, tag=tag)
                bc = bass.AP(tensor=src_ap.tensor, offset=src_ap.offset,
                             ap=[[0, P]] + list(src_ap.ap))
                nc.gpsimd.dma_start(out=t, in_=bc)
                return t

            if n1_aff:
                n1w_sb = bcast_load(n1w_d[:], C, "n1w")
                n1b_sb = bcast_load(n1b_d[:], C, "n1b")
            if n2_aff:
                n2w_sb = bcast_load(n2w_d[:], C, "n2w")
                n2b_sb = bcast_load(n2b_d[:], C, "n2b")
            if hln_aff:
                hlnw_sb = bcast_load(hlnw_d[:], HD, "hlnw")
                hlnb_sb = bcast_load(hlnb_d[:], HD, "hlnb")

            # -------- helpers --------
            def layernorm_chunk(x_tile, out_tile, w_sb, b_sb):
                """LN over free dim 768 of a [128, 768] tile (fp32)."""
                st = s2.tile([P, 3, 6], f32, tag="lnst")
                for g in range(3):
                    nc.vector.bn_stats(st[:, g, :], x_tile[:, g * 256:(g + 1) * 256])
                mv = s2.tile([P, 2], f32, tag="lnmv")
                nc.vector.bn_aggr(mv, st)
                std = s2.tile([P, 1], f32, tag="lnstd")
                nc.scalar.activation(std, mv[:, 1:2], AF.Sqrt, bias=epsC)
                rstd = s2.tile([P, 1], f32, tag="lnrstd")
                nc.vector.reciprocal(rstd, std)
                nc.vector.tensor_scalar(out_tile, x_tile, mv[:, 0:1], rstd,
                                        ALU.subtract, ALU.mult)
                if w_sb is not None:
                    nc.vector.tensor_tensor(out_tile, out_tile, w_sb, ALU.mult)
                    nc.vector.tensor_tensor(out_tile, out_tile, b_sb, ALU.add)

            # ======== P1 + QKV, streams interleaved per chunk ========
            xnTd = {}
            for s in "ba":
                xnTd[s] = big.tile([P, KC, N], bf16, name=f"xnT_{s}", tag="big")
            wq = s1.tile([P, KC, S3], bf16, tag="wbig")
            nc.sync.dma_start(wq, qkv_wT[:].rearrange("(k p) f -> p k f", p=P))
            pw2sb = s1.tile([P, NJ, C], bf16, tag="pw2sb")
            nc.sync.dma_start(pw2sb, pw2_d[:].rearrange("p (j o) -> p j o", o=C))

            QSL = [(0, 1024), (1024, 1024), (2048, 256)]

            for c in range(NCH):
                cs = slice(c * P, (c + 1) * P)
                for s in "ba":
                    with nc.named_scope(f"p1_{s}"):
                        xt = sF.tile([P, C], f32, tag="f32buf", name="xt")
                        nc.sync.dma_start(xt, x_in[s][cs, :])
                        xn = sF.tile([P, C], f32, tag="f32buf", name="xn")
                        layernorm_chunk(xt, xn,
                                        n1w_sb if n1_aff else None,
                                        n1b_sb if n1_aff else None)
                        nc.sync.dma_start(xn_t[s][cs, :], xn)
                        xnb = s2.tile([P, C], bf16, tag="xnb")
                        nc.scalar.copy(xnb, xn)
                        tp = ps.tile([P, KC, P], bf16, tag="A", name="tp1")
                        for t in range(KC):
                            nc.tensor.transpose(tp[:, t, :], xnb[:, t * P:(t + 1) * P], ident)
                        nc.vector.tensor_copy(xnTd[s][:, :, cs], tp)

                for s in "ba":
                    with nc.named_scope(f"qkv_{s}"):
                        accs = []
                        for i, (f0, fw) in enumerate(QSL):
                            acc = ps.tile([P, fw], f32, tag="A", name=f"qacc{i}")
                            for k in range(KC):
                                for m0 in range(0, fw, 512):
                                    mw = min(512, fw - m0)
                                    nc.tensor.matmul(
                                        acc[:, m0:m0 + mw],
                                        xnTd[s][:, k, cs],
                                        wq[:, k, f0 + m0:f0 + m0 + mw],
                                        start=(k == 0), stop=(k == KC - 1))
                            accs.append(acc)
                        # squares -> sumsq -> rstd  (centering folded into W')
                        sumsq = s2.tile([P, NG], f32, tag="hsumsq")
                        for i, (f0, fw) in enumerate(QSL):
                            sq = s2.tile([P, 1024], bf16, tag="sq")
                            nc.scalar.activation(sq[:, :fw], accs[i], AF.Square)
                            nc.vector.reduce_sum(
                                sumsq[:, f0 // HD:(f0 + fw) // HD],
                                sq[:, :fw].rearrange("p (g d) -> p g d", d=HD),
                                axis=AX)
                        stdq = s2.tile([P, NG], f32, tag="hstd")
                        nc.scalar.activation(stdq, sumsq, AF.Sqrt, bias=epsC,
                                             scale=1.0 / HD)
                        rstd = s2.tile([P, NG], f32, tag="hrstd")
                        nc.vector.reciprocal(rstd, stdq)
                        # normalize (+ optional affine) straight out of PSUM -> bf16
                        zb = s2.tile([P, S3], bf16, tag="zb")
                        for i, (f0, fw) in enumerate(QSL):
                            g0 = f0 // HD
                            gw = fw // HD
                            zv = zb[:, f0:f0 + fw].rearrange("p (g d) -> p g d", d=HD)
                            nc.vector.tensor_tensor(
                                zv,
                                accs[i].rearrange("p (g d) -> p g d", d=HD),
                                rstd[:, g0:g0 + gw, None].to_broadcast([P, gw, HD]),
                                ALU.mult)
                        if hln_aff:
                            z3 = zb.rearrange("p (g d) -> p g d", d=HD)
                            nc.vector.tensor_tensor(
                                z3, z3, hlnw_sb[:, None, :].to_broadcast([P, NG, HD]),
                                ALU.mult)
                            nc.vector.tensor_tensor(
                                z3, z3, hlnb_sb[:, None, :].to_broadcast([P, NG, HD]),
                                ALU.add)
                        # transposes of q,k sections (12 x 128)
                        for half in range(2):
                            tp2 = ps.tile([P, KC, P], bf16, tag="A", name="tp2")
                            for t in range(KC):
                                tt = half * KC + t
                                nc.tensor.transpose(
                                    tp2[:, t, :], zb[:, tt * P:(tt + 1) * P], ident)
                            qkt_sb = s2.tile([P, KC, P], bf16, tag="qkt")
                            nc.vector.tensor_copy(qkt_sb, tp2)
                            nc.sync.dma_start(
                                qkT_t[s][:].rearrange("(t p) n -> p t n", p=P)
                                [:, half * KC:(half + 1) * KC, cs],
                                qkt_sb)
                        nc.gpsimd.dma_start(v_t[s][cs, :], zb[:, 2 * C:])
                        nc.sync.dma_start(
                            qr_t[s][:].rearrange("(h n) d -> n h d", h=H)[cs],
                            zb[:, :C].rearrange("p (g d) -> p g d", d=HD))

            # ======== Attention: both directions, head interleaved ========
            DIRS = (("b", "a"), ("a", "b"))  # (qs, ks); output goes to stream ks
            ctx2 = {}
            for qs, ks in DIRS:
                ctx2[qs] = big.tile([P, NJ, N], bf16, name=f"ctx2_{qs}", tag="big")
            # persistent [v | ones] stationaries: parity x direction
            vap = {}
            for hp in range(2):
                for qs, ks in DIRS:
                    t = s1.tile([P, NCH, P], f8, tag=f"va{hp}{qs}")
                    nc.gpsimd.memset(t[:, :, (1 - hp) * HD:(2 - hp) * HD], 1.0)
                    vap[(hp, qs)] = t

            for j in range(NJ):
                for hp in range(2):
                    h = 2 * j + hp
                    hs = slice(hp * HD, (hp + 1) * HD)        # ctx half
                    ds = slice((1 - hp) * HD, (2 - hp) * HD)  # denominator half
                    lo = slice(0, HD)
                    for qs, ks in DIRS:
                        with nc.named_scope(f"attn_{qs}"):
                            qt = s3b.tile([HD, N], bf16, tag="qh",
                                          name=f"qh_{qs}{h}")
                            nc.sync.dma_start(
                                qt, qkT_t[qs][h * HD:(h + 1) * HD, :])
                            kt = s3b.tile([HD, N], bf16, tag="kh",
                                          name=f"kh_{qs}{h}")
                            nc.sync.dma_start(
                                kt, qkT_t[ks][C + h * HD:C + (h + 1) * HD, :])
                            va = vap[(hp, qs)]
                            nc.sync.dma_start(
                                va[:, :, hp * HD:(hp + 1) * HD],
                                v_t[ks][:].rearrange("(c p) f -> p c f", p=P)
                                [:, :, h * HD:(h + 1) * HD])
                            cps = ps.tile([P, 2, 512], f32, tag="A", name="cps")
                            for mc2 in range(NCH // 2):
                                pt2 = s3b.tile([P, 2, 2, 512], f8, tag="pt")
                                for mi in range(2):
                                    mc = 2 * mc2 + mi
                                    sps = ps.tile([P, 2, 512], f32, tag="A",
                                                  name="sps")
                                    for nh in range(2):
                                        nc.tensor.matmul(
                                            sps[:, nh, :],
                                            kt[:, mc * P:(mc + 1) * P],
                                            qt[:, nh * 512:(nh + 1) * 512])
                                    nc.scalar.activation(
                                        pt2[:, mi].rearrange("p a b -> p (a b)"),
                                        sps.rearrange("p a b -> p (a b)"),
                                        AF.Exp, scale=float(HD ** -0.5), bias=eln)
                                for nh in range(2):
                                    nc.tensor.matmul(
                                        cps[:, nh, :],
                                        va[:, 2 * mc2:2 * mc2 + 2, :],
                                        pt2[:, :, nh, :],
                                        perf_mode=DR,
                                        start=(mc2 == 0), stop=(mc2 == NCH // 2 - 1))
                            # denominator (replicated on partitions ds):
                            # aligned copy out of PSUM, shift to base 0,
                            # recipfast at base 0, shift to hs, aligned mult.
                            dn = s3.tile([P, N], f32, tag="dn")
                            nc.vector.tensor_copy(
                                dn[ds, :],
                                cps[ds, :, :].rearrange("p a b -> p (a b)"))
                            if hp == 0:
                                nc.scalar.dma_start(dn[lo, :], dn[ds, :])
                            rd = s3.tile([P, N], f32, tag="rd")
                            nc.vector.reciprocal_approx_fast(rd[lo, :], dn[lo, :])
                            if hp == 1:
                                nc.scalar.dma_start(rd[hs, :], rd[lo, :])
                            nc.vector.tensor_tensor(
                                ctx2[qs][hs, j, :],
                                cps[hs, :, :].rearrange("p a b -> p (a b)"),
                                rd[hs, :], ALU.mult)

            # ======== proj + residual + LN2 per direction ========
            for qs, ks in DIRS:
                with nc.named_scope(f"proj_{ks}"):
                    qr_view = qr_t[qs][:].rearrange("(n j) d -> n (j d)", j=H)
                    for c in range(NCH):
                        cs = slice(c * P, (c + 1) * P)
                        y = ps.tile([P, C], f32, tag="A", name="yproj")
                        for jj in range(NJ):
                            for o0, ow in ((0, 512), (512, 256)):
                                nc.tensor.matmul(
                                    y[:, o0:o0 + ow],
                                    ctx2[qs][:, jj, cs],
                                    pw2sb[:, jj, o0:o0 + ow],
                                    start=(jj == 0),
                                    stop=(jj == NJ - 1 and not has_projb))
                        if has_projb:
                            for o0, ow in ((0, 512), (512, 256)):
                                nc.tensor.matmul(
                                    y[:, o0:o0 + ow], ones_bf[0:1, :],
                                    projb_sb[0:1, o0:o0 + ow],
                                    start=False, stop=True)
                        xnr = sF.tile([P, C], f32, tag="f32buf", name="xnr")
                        nc.sync.dma_start(xnr, xn_t[ks][cs, :])
                        qres = s2.tile([P, C], bf16, tag="qres")
                        nc.sync.dma_start(qres, qr_view[cs, :])
                        ot = sF.tile([P, C], f32, tag="f32buf", name="ot")
                        nc.vector.tensor_tensor(ot, y, xnr, ALU.add)
                        nc.vector.tensor_tensor(ot, ot, qres, ALU.add)
                        nc.sync.dma_start(o_t[ks][cs, :], ot)
                        x2 = sF.tile([P, C], f32, tag="f32buf", name="x2")
                        layernorm_chunk(ot, x2,
                                        n2w_sb if n2_aff else None,
                                        n2b_sb if n2_aff else None)
                        x2b = s2.tile([P, C], bf16, tag="xnb")
                        nc.scalar.copy(x2b, x2)
                        tp3 = ps.tile([P, KC, P], bf16, tag="A", name="tp3")
                        for t in range(KC):
                            nc.tensor.transpose(tp3[:, t, :], x2b[:, t * P:(t + 1) * P],
                                                ident)
                        x2ts = s2.tile([P, KC, P], bf16, tag="x2ts")
                        nc.vector.tensor_copy(x2ts, tp3)
                        nc.sync.dma_start(
                            x2T_t[ks][:].rearrange("(t p) n -> p t n", p=P)[:, :, cs],
                            x2ts)

            # ======== MLP per stream, fp8 DoubleRow ========
            w2sb = s1.tile([P, HKC, C], f8, tag="wbig")
            nc.sync.dma_start(w2sb, w2p_d[:].rearrange("p (k o) -> p k o", o=C))
            w1v = w1p_d[:].rearrange("p (kc k f) -> p kc k f", k=KC, f=P)
            for s in "ab":
                with nc.named_scope(f"mlp_{s}"):
                    x2h = []
                    for nh in range(2):
                        xh = sB.tile([P, KC, 512], f8, tag="x2h", name=f"x2h_{s}{nh}")
                        nc.gpsimd.dma_start(
                            out=xh,
                            in_=x2T_t[s][:].rearrange("(k p) n -> p k n", p=P)
                            [:, :, nh * 512:(nh + 1) * 512])
                        x2h.append(xh)
                    hT = []
                    for nh in range(2):
                        hT.append(big.tile([P, HKC, 512], f8, tag="big",
                                           name=f"hT_{s}{nh}"))
                    for kc2 in range(HKC // 2):
                        w1k = []
                        for kk in range(2):
                            w1t = s3b.tile([P, KC, P], f8, tag="w1k")
                            nc.scalar.dma_start(w1t, w1v[:, 2 * kc2 + kk, :, :])
                            w1k.append(w1t)
                        accs = [ps.tile([P, 2, 512], f32, tag="A", name=f"facc{nh}")
                                for nh in range(2)]
                        for kp in range(0, KC, 2):
                            for kk in range(2):
                                for nh in range(2):
                                    nc.tensor.matmul(
                                        accs[nh][:, kk, :],
                                        w1k[kk][:, kp:kp + 2, :],
                                        x2h[nh][:, kp:kp + 2, :],
                                        perf_mode=DR,
                                        start=(kp == 0), stop=(kp == KC - 2))
                        for nh in range(2):
                            if has_fc1b:
                                for kk in range(2):
                                    kc = 2 * kc2 + kk
                                    nc.scalar.activation(
                                        hT[nh][:, kc, :], accs[nh][:, kk, :],
                                        AF.Gelu, bias=fc1b_sb[:, kc:kc + 1],
                                        scale=1.0 / W8SCALE)
                            else:
                                nc.scalar.activation(
                                    hT[nh][:, 2 * kc2:2 * kc2 + 2, :]
                                    .rearrange("p a b -> p (a b)"),
                                    accs[nh].rearrange("p a b -> p (a b)"),
                                    AF.Gelu, scale=1.0 / W8SCALE)
                    for nh in range(2):
                        for sub in range(4):
                            c = nh * 4 + sub
                            cs = slice(c * P, (c + 1) * P)
                            y = ps.tile([P, C], f32, tag="A", name="yfc2")
                            for kc in range(0, HKC, 2):
                                for o0, ow in ((0, 512), (512, 256)):
                                    nc.tensor.matmul(
                                        y[:, o0:o0 + ow],
                                        hT[nh][:, kc:kc + 2, sub * P:(sub + 1) * P],
                                        w2sb[:, kc:kc + 2, o0:o0 + ow],
                                        perf_mode=DR,
                                        start=(kc == 0),
                                        stop=(kc == HKC - 2 and not has_fc2b))
                            if has_fc2b:
                                for o0, ow in ((0, 512), (512, 256)):
                                    nc.tensor.matmul(
                                        y[:, o0:o0 + ow], ones_bf[0:1, :],
                                        fc2b_sb[0:1, o0:o0 + ow],
                                        start=False, stop=True)
                            oh = sF.tile([P, C], f32, tag="f32buf", name="oh")
                            nc.sync.dma_start(oh, o_t[s][cs, :])
                            outt = sF.tile([P, C], f32, tag="f32buf", name="outt")
                            nc.vector.scalar_tensor_tensor(
                                outt, y, 1.0 / W8SCALE, oh, ALU.mult, ALU.add)
                            nc.sync.dma_start(out_d[s][cs, :], outt)

    nc.finalize()
    return nc


def _get_nc(flags):
    if flags not in _CACHE:
        _CACHE[flags] = _build(flags)
    return _CACHE[flags]


def _prep(inputs):
    import ml_dtypes

    f = np.float32
    bf = ml_dtypes.bfloat16
    f8 = ml_dtypes.float8_e4m3
    w = {k: np.asarray(v, f) for k, v in inputs.items()}
    flags = (
        not (np.all(w["norm1_w"] == 1) and np.all(w["norm1_b"] == 0)),
        not (np.all(w["hln_w"] == 1) and np.all(w["hln_b"] == 0)),
        not (np.all(w["norm2_w"] == 1) and np.all(w["norm2_b"] == 0)),
        bool(np.any(w["proj_b"] != 0)),
        bool(np.any(w["fc1_b"] != 0)),
        bool(np.any(w["fc2_b"] != 0)),
    )
    # qkv weights: transpose + fold head-LN centering (linear in x)
    wT = np.ascontiguousarray(w["qkv_w"].T)                   # [C, 3C]
    wT3 = wT.reshape(C, NG, HD)
    wTc = (wT3 - wT3.mean(axis=2, keepdims=True)).reshape(C, S3)
    # proj weights packed by head pair: pw2[p=(h%2)*64+d, j=h//2, o]
    pw = w["proj_w"].T.reshape(NJ, 2, HD, C).transpose(1, 2, 0, 3).reshape(P, NJ * C)
    # fc1 packed: w1p[p, kc, k, f'] = 32*fc1_w[kc*128+f', k*128+p], fp8
    w1p = (W8SCALE * w["fc1_w"]).reshape(HKC, P, KC, P).transpose(3, 0, 2, 1).reshape(P, HKC * C)
    # fc2 packed: w2p[p, kc, o] = 32*fc2_w[o, kc*128+p], fp8
    w2p = (W8SCALE * w["fc2_w"]).reshape(C, HKC, P).transpose(2, 1, 0).reshape(P, HKC * C)
    shared = {
        "qkv_wT": wTc.astype(bf),
        "pw2": np.ascontiguousarray(pw).astype(bf),
        "w1p": np.ascontiguousarray(w1p).astype(f8),
        "w2p": np.ascontiguousarray(w2p).astype(f8),
    }
    n1_aff, hln_aff, n2_aff, pb, f1b, f2b = flags
    if pb:
        shared["proj_b"] = w["proj_b"].reshape(1, C).astype(bf)
    if f1b:
        shared["fc1_b"] = w["fc1_b"]
    if f2b:
        shared["fc2_b"] = (w["fc2_b"] * W8SCALE).reshape(1, C).astype(bf)
    if n1_aff:
        shared["norm1_w"] = w["norm1_w"]
        shared["norm1_b"] = w["norm1_b"]
    if n2_aff:
        shared["norm2_w"] = w["norm2_w"]
        shared["norm2_b"] = w["norm2_b"]
    if hln_aff:
        shared["hln_w"] = w["hln_w"]
        shared["hln_b"] = w["hln_b"]
    return w, flags, shared


def kernel(trace=False, **inputs):
    from concourse.bass_utils import run_bass_kernel_spmd

    w, flags, shared = _prep(inputs)
    nc = _get_nc(flags)
    before = np.ascontiguousarray(w["before"], dtype=np.float32)
    after = np.ascontiguousarray(w["after"], dtype=np.float32)
    in_maps = []
    for core in range(B):
        m = dict(shared)
        m["x_b"] = np.ascontiguousarray(before[core])
        m["x_a"] = np.ascontiguousarray(after[core])
        in_maps.append(m)
    res = run_bass_kernel_spmd(nc, in_maps, core_ids=list(range(B)), trace=trace)
    before_o = np.stack([res.results[i]["out_b"] for i in range(B)])
    after_o = np.stack([res.results[i]["out_a"] for i in range(B)])
    out = (before_o.astype(np.float32), after_o.astype(np.float32))
    if trace:
        return out, res
    return out
